# revision 1
# baseline (speedup 1.0000x reference)
"""GRU scan kernel for Trainium2, 8-core data-parallel.

Problem: B=64, S=512, I=512, H=1024, O=2 GRU + FC + log_softmax.
Strategy: shard batch 8-way (8 rows/core). Per core:
  1. Precompute xpart[b,t,:] = x[b,t,:] @ Wx_all + b_all  (fp32r matmuls,
     [r|z|hc] column order), store bf16 in internal DRAM.
  2. 512-step scan. Per step, recurrent matmuls in bf16 with a packed
     layout: preacts computed as 16 chunks of [8 batch, 128 feat] placed
     at psum partitions 8c so four PE column-groups run concurrently.
     Elementwise work runs densely packed on 128 (z,r) / 64 (hc)
     partitions. h^T (the matmul stationary operand) is maintained via a
     PE transpose of the packed h each step.
  3. FC head + log_softmax on device.
"""

import os
import sys
from contextlib import ExitStack

for _p in ("/opt/trn_rl_repo",):
    if os.path.isdir(_p) and _p not in sys.path:
        sys.path.insert(0, _p)

import numpy as np
import ml_dtypes

import concourse.bass as bass
import concourse.mybir as mybir
import concourse.tile as tile
from concourse import bacc
from concourse.bass import ds
from concourse.bass_utils import run_bass_kernel_spmd

B, S, I, H, O = 64, 512, 512, 1024, 2
NCORES = 8
BL = B // NCORES          # 8 batch rows per core
G3 = 3 * H                # 3072 gate features, order [r | z | hc]
KT = H // 128             # 8 k-tiles over hidden dim
F32, F32R, BF16 = mybir.dt.float32, mybir.dt.float32r, mybir.dt.bfloat16
AFT = mybir.ActivationFunctionType


def _lhsT(q, k, w=8):
    """[128, w] stationary slice for hidden k-tile k from a Q-layout tile.

    Q layout [128, 2, 128]: entry [p, u, 32j+b] = v[b, 256j + 128u + p].
    k-tile k (hidden feats [128k, 128k+128)) -> u = k%2, cols 32*(k//2)+[0,w).
    Columns 8..w hold other k-tiles' data; the junk output rows they produce
    keep the full psum block initialized.
    """
    j, u = divmod(k, 2)
    return q[:, u, 32 * j : 32 * j + w]


def _scan_step(nc, tc, pools, consts, step_ap):
    """Emit one GRU step. step_ap indexes xpart's time dim (dynamic)."""
    (ptmp, ppsum, ptps, pxp) = pools
    (whh, id8w, id128, hQ, hQbf) = consts

    xp = pxp.tile([BL, 1, G3], BF16, tag="xp")
    nc.sync.dma_start(out=xp, in_=step_ap)
    xpf = xp[:, 0, :]

    # ---- phase A: r and z preacts. chunk j covers gate feats [256j, 256j+256)
    # at psum partitions [32*(j%4), +8). r = chunks 0-3 (tile A), z = 4-7 (B).
    r_ps = ppsum.tile([128, 256], F32, tag="r_ps")
    z_ps = ppsum.tile([128, 256], F32, tag="z_ps")

    def gate_mm(ps, j, k):
        b0 = 32 * (j % 4)
        out = ps[b0 : b0 + 32, :]
        if k < 0:  # xpart via identity matmul
            nc.tensor.matmul(out, id8w, xpf[:, 256 * j : 256 * j + 256],
                             start=True, stop=False, tile_position=(0, b0),
                             skip_group_check=True)
        else:
            nc.tensor.matmul(out, _lhsT(hQbf, k, 32),
                             whh[:, G3 * k + 256 * j : G3 * k + 256 * j + 256],
                             start=False, stop=(k == KT - 1),
                             tile_position=(0, b0), skip_group_check=True)

    # r chunks first (4-way), then z chunks (4-way) -> r-half finishes early
    for k in range(-1, KT):
        for j in range(4):
            gate_mm(r_ps, j, k)
    for k in range(-1, KT):
        for j in range(4, 8):
            gate_mm(z_ps, j, k)

    r_sb = ptmp.tile([128, 256], F32, tag="r_sb")
    nc.scalar.activation(r_sb, r_ps, AFT.Sigmoid)
    z_sb = ptmp.tile([128, 256], F32, tag="z_sb")
    nc.scalar.activation(z_sb, z_ps, AFT.Sigmoid)

    # transpose sigmoided preacts into Q space
    rQ = ptmp.tile([128, 2, 128], F32, tag="rQ")
    zQ = ptmp.tile([128, 2, 128], F32, tag="zQ")
    for u in range(2):
        tp = ptps.tile([128, 128], F32, tag="tp")
        nc.tensor.transpose(tp, r_sb[:, 128 * u : 128 * (u + 1)], id128)
        nc.vector.tensor_copy(rQ[:, u, :], tp)
    for u in range(2):
        tp = ptps.tile([128, 128], F32, tag="tp")
        nc.tensor.transpose(tp, z_sb[:, 128 * u : 128 * (u + 1)], id128)
        nc.scalar.copy(zQ[:, u, :], tp)

    # rh^T directly in Q space (bf16 out feeds the hc matmul)
    rhQ = ptmp.tile([128, 2, 128], BF16, tag="rhQ")
    nc.vector.tensor_mul(rhQ, rQ, hQ)

    # ---- phase B: hc preact. 4 chunks of 256 at the 4 bases (4-way).
    hc_ps = ppsum.tile([128, 256], F32, tag="hc_ps")
    for j in range(4):
        b0 = 32 * j
        nc.tensor.matmul(hc_ps[b0 : b0 + 32, :], id8w,
                         xpf[:, 2048 + 256 * j : 2048 + 256 * j + 256],
                         start=True, stop=False, tile_position=(0, b0),
                         skip_group_check=True)
    for k in range(KT):
        for j in range(4):
            b0 = 32 * j
            nc.tensor.matmul(
                hc_ps[b0 : b0 + 32, :], _lhsT(rhQ, k, 32),
                whh[:, G3 * k + 2048 + 256 * j : G3 * k + 2048 + 256 * j + 256],
                start=False, stop=(k == KT - 1),
                tile_position=(0, b0), skip_group_check=True)

    hc_sb = ptmp.tile([128, 256], F32, tag="hc_sb")
    nc.scalar.activation(hc_sb, hc_ps, AFT.Tanh)
    hcQ = ptmp.tile([128, 2, 128], F32, tag="hcQ")
    for u in range(2):
        tp = ptps.tile([128, 128], F32, tag="tp")
        nc.tensor.transpose(tp, hc_sb[:, 128 * u : 128 * (u + 1)], id128)
        nc.scalar.activation(hcQ[:, u, :], tp, AFT.Copy)

    # blend in Q space: h = h + z*(hc - h)
    d1 = ptmp.tile([128, 2, 128], F32, tag="d1")
    nc.vector.tensor_sub(d1, hcQ, hQ)
    d2 = ptmp.tile([128, 2, 128], F32, tag="d2")
    nc.vector.tensor_mul(d2, zQ, d1)
    nc.vector.tensor_add(hQ, hQ, d2)
    nc.vector.tensor_copy(hQbf, hQ)


def build(nc_s=S, unroll=8, use_for_i=True, scan_reps=1):
    """Build the Bass program. nc_s = number of scan steps (512 for real)."""
    nc = bacc.Bacc("TRN2", target_bir_lowering=False, debug=False,
                   num_devices=NCORES)

    xt_d = nc.dram_tensor("xt", [I, BL * nc_s], F32R, kind="ExternalInput")
    h0q_d = nc.dram_tensor("h0q", [128, 2, 128], F32, kind="ExternalInput")
    wx_d = nc.dram_tensor("wx", [I, G3], F32R, kind="ExternalInput")
    bias_d = nc.dram_tensor("bias", [1, G3], F32R, kind="ExternalInput")
    whh_d = nc.dram_tensor("whh", [H, G3], BF16, kind="ExternalInput")
    wfc_d = nc.dram_tensor("wfc", [H, O], F32R, kind="ExternalInput")
    bfc_d = nc.dram_tensor("bfc", [1, O], F32R, kind="ExternalInput")
    id128f_d = nc.dram_tensor("id128f", [128, 128], F32, kind="ExternalInput")
    id8_d = nc.dram_tensor("id8", [8, 32], BF16, kind="ExternalInput")
    ones_d = nc.dram_tensor("ones", [1, 128], F32R, kind="ExternalInput")
    out_d = nc.dram_tensor("out", [BL, O], F32, kind="ExternalOutput")

    xpart_d = nc.dram_tensor("xpart", [BL, nc_s, G3], BF16)

    n_rows = BL * nc_s
    n_rt = (n_rows + 127) // 128

    with tile.TileContext(nc) as tc, ExitStack() as ctx:
        # ---------------- constants resident in SBUF ----------------
        pconst = ctx.enter_context(tc.tile_pool(name="pconst", bufs=1))
        whh = pconst.tile([128, KT * G3], BF16)
        for k in range(KT):
            nc.sync.dma_start(out=whh[:, G3 * k : G3 * (k + 1)],
                              in_=whh_d[128 * k : 128 * (k + 1), :])
        wx = pconst.tile([128, 4 * G3], F32R)
        for k in range(4):
            nc.sync.dma_start(out=wx[:, G3 * k : G3 * (k + 1)],
                              in_=wx_d[128 * k : 128 * (k + 1), :])
        bias_sb = pconst.tile([1, G3], F32R)
        nc.sync.dma_start(out=bias_sb, in_=bias_d[:, :])
        id128 = pconst.tile([128, 128], F32)
        nc.sync.dma_start(out=id128, in_=id128f_d[:, :])
        id8w = pconst.tile([8, 32], BF16)
        nc.sync.dma_start(out=id8w, in_=id8_d[:, :])
        ones128 = pconst.tile([1, 128], F32R)
        nc.sync.dma_start(out=ones128, in_=ones_d[:, :])
        wfc = pconst.tile([128, KT * O], F32R)
        for k in range(KT):
            nc.sync.dma_start(out=wfc[:, O * k : O * (k + 1)],
                              in_=wfc_d[128 * k : 128 * (k + 1), :])
        bfc_sb = pconst.tile([1, O], F32R)
        nc.sync.dma_start(out=bfc_sb, in_=bfc_d[:, :])

        # persistent scan state (Q layout)
        hQ = pconst.tile([128, 2, 128], F32)
        nc.sync.dma_start(out=hQ, in_=h0q_d[:, :, :])
        hQbf = pconst.tile([128, 2, 128], BF16)
        nc.vector.tensor_copy(hQbf, hQ)

        # ---------------- precompute xpart ----------------
        ppre = ctx.enter_context(tc.tile_pool(name="ppre", bufs=2))
        ppre_ps = ctx.enter_context(tc.tile_pool(name="ppre_ps", bufs=2,
                                                 space="PSUM"))
        xpart_flat = xpart_d.rearrange("b s f -> (b s) f")
        for rt in range(n_rt):
            r0 = rt * 128
            xt_sb = ppre.tile([128, 4, 128], F32R, tag="xt_sb")
            for k in range(4):
                nc.sync.dma_start(
                    out=xt_sb[:, k, :],
                    in_=xt_d[128 * k : 128 * (k + 1), r0 : r0 + 128])
            xp_sb = ppre.tile([128, G3], BF16, tag="xp_sb")
            for n in range(6):
                xp_ps = ppre_ps.tile([128, 512], F32, tag="xp_ps")
                ns = slice(512 * n, 512 * (n + 1))
                nc.tensor.matmul(xp_ps, ones128, bias_sb[:, ns],
                                 start=True, stop=False)
                for k in range(4):
                    nc.tensor.matmul(
                        xp_ps, xt_sb[:, k, :],
                        wx[:, G3 * k + 512 * n : G3 * k + 512 * (n + 1)],
                        start=False, stop=(k == 3))
                if n % 2 == 1:
                    nc.scalar.copy(xp_sb[:, ns], xp_ps)
                else:
                    nc.vector.tensor_copy(xp_sb[:, ns], xp_ps)
            nc.sync.dma_start(out=xpart_flat[r0 : r0 + 128, :], in_=xp_sb)

        # ---------------- scan ----------------
        ptmp = ctx.enter_context(tc.tile_pool(name="ptmp", bufs=1))
        ppsum = ctx.enter_context(tc.tile_pool(name="ppsum", bufs=1, space="PSUM"))
        ptps = ctx.enter_context(tc.tile_pool(name="ptps", bufs=3, space="PSUM"))
        pxp = ctx.enter_context(tc.tile_pool(name="pxp", bufs=3))
        pools = (ptmp, ppsum, ptps, pxp)
        consts = (whh, id8w, id128, hQ, hQbf)

        for _rep in range(scan_reps):
            if use_for_i:
                assert nc_s % unroll == 0
                with tc.For_i(0, nc_s, unroll) as iv:
                    for u in range(unroll):
                        _scan_step(nc, tc, pools, consts,
                                   xpart_d[:, ds(iv + u, 1), :])
            else:
                for t in range(nc_s):
                    _scan_step(nc, tc, pools, consts,
                               xpart_d[:, t : t + 1, :])

        # ---------------- FC head + log_softmax ----------------
        hrelu = ptmp.tile([128, 2, 128], F32R, tag="hrelu")
        nc.scalar.activation(hrelu, hQ, AFT.Relu)

        fc_ps = ptps.tile([BL, O], F32, tag="tp")
        nc.tensor.matmul(fc_ps, ones128[:, :BL], bfc_sb, start=True, stop=False)
        for k in range(KT):
            nc.tensor.matmul(fc_ps, _lhsT(hrelu, k),
                             wfc[:, O * k : O * (k + 1)],
                             start=False, stop=(k == KT - 1))

        mx = ptmp.tile([BL, 1], F32, tag="mx")
        nc.vector.tensor_reduce(mx, fc_ps, mybir.AxisListType.X,
                                mybir.AluOpType.max)
        tt = ptmp.tile([BL, O], F32, tag="tt")
        nc.vector.tensor_scalar(tt, fc_ps, mx, None, mybir.AluOpType.subtract)
        ex = ptmp.tile([BL, O], F32, tag="ex")
        nc.scalar.activation(ex, tt, AFT.Exp)
        sm = ptmp.tile([BL, 1], F32, tag="sm")
        nc.vector.tensor_reduce(sm, ex, mybir.AxisListType.X,
                                mybir.AluOpType.add)
        lsm = ptmp.tile([BL, 1], F32, tag="lsm")
        nc.scalar.activation(lsm, sm, AFT.Ln)
        res = ptmp.tile([BL, O], F32, tag="res")
        nc.vector.tensor_scalar(res, tt, lsm, None, mybir.AluOpType.subtract)
        nc.sync.dma_start(out=out_d[:, :], in_=res)

    nc.compile()
    return nc


def prep_inputs(x, h, Wz, bz, Wr, br, Wh, bh, Wfc, bfc, nc_s=S):
    """Host-side prep: shard + relayout. Returns per-core input maps."""
    f32 = np.float32
    x = np.asarray(x, f32)[:, :nc_s, :]
    h0 = np.asarray(h, f32)[:, 0, :]
    Wx_all = np.concatenate([np.asarray(Wr, f32)[:I], np.asarray(Wz, f32)[:I],
                             np.asarray(Wh, f32)[:I]], axis=1)
    b_all = np.concatenate([np.asarray(br, f32), np.asarray(bz, f32),
                            np.asarray(bh, f32)])[None, :]
    Whh_all = np.concatenate([np.asarray(Wr, f32)[I:], np.asarray(Wz, f32)[I:],
                              np.asarray(Wh, f32)[I:]], axis=1)
    Whh_bf = Whh_all.astype(ml_dtypes.bfloat16)
    id128 = np.eye(128, dtype=f32)
    id8 = np.zeros((8, 32), ml_dtypes.bfloat16)
    np.fill_diagonal(id8[:, :8], 1)
    wfc = np.asarray(Wfc, f32)
    bfc_a = np.asarray(bfc, f32)[None, :]

    in_maps = []
    for c in range(NCORES):
        xc = x[c * BL : (c + 1) * BL]                      # [8, S, I]
        xt = xc.reshape(BL * nc_s, I).T.copy()             # [I, 8*S]
        h0c = h0[c * BL : (c + 1) * BL]                    # [8, H]
        # Q layout: h0q[p, u, 32j+b] = h0c[b, 256j + 128u + p]
        h0q = np.zeros((128, 2, 128), f32)
        hv = h0c.reshape(BL, 4, 2, 128)                    # [b, j, u, p]
        for j in range(4):
            h0q[:, :, 32 * j : 32 * j + BL] = hv[:, j].transpose(2, 1, 0)
        in_maps.append({
            "xt": xt, "h0q": h0q,
            "wx": Wx_all, "bias": b_all, "whh": Whh_bf,
            "wfc": wfc, "bfc": bfc_a,
            "id128f": id128, "id8": id8, "ones": np.ones((1, 128), f32),
        })
    return in_maps


_BUILT = {}
_LAST_RESULTS = None


def kernel(**inputs):
    global _LAST_RESULTS
    key = "full"
    if key not in _BUILT:
        _BUILT[key] = build(S, unroll=8, use_for_i=True)
    nc = _BUILT[key]
    in_maps = prep_inputs(**inputs)
    trace = bool(int(os.environ.get("BASS_TRACE", "0") or "0"))
    res = run_bass_kernel_spmd(nc, in_maps, list(range(NCORES)), trace=trace)
    _LAST_RESULTS = res
    outs = [res.results[c]["out"] for c in range(NCORES)]
    return np.concatenate(outs, axis=0).astype(np.float32)


if __name__ == "__main__":
    np.random.seed(0)
    print("building...")
    nc = build(16, unroll=8, use_for_i=True)
    print("build ok:", nc)



# revision 6
# speedup vs baseline: 1.5547x; 1.5547x over previous
"""GRU scan kernel for Trainium2, 8-core data-parallel.

Problem: B=64, S=512, I=512, H=1024, O=2 GRU + FC + log_softmax.

Strategy (v2): shard batch 8-way (8 rows/core). Per core, a 512-step scan
where each step streams Whh (bf16, [1024, 3072]) through the PE at 4-way
column-group concurrency (tile_position), with batch-8 stationaries.

Key layout: the "staircase" SM/ST pair, chosen so SM -> ST is exactly the
DVE's 32x32-block transpose (nc.vector.transpose):
  SM[32g+b, 32m+i] = v[b, 128m+32g+i]   (batch-major, for elementwise)
  ST[32g+i, 32m+b] = v[b, 128m+32g+i]   (feature-major; ST[:, 32k:32k+32]
                                          is the matmul stationary for
                                          contraction k-tile k)
Weights are column-permuted on the host so gate matmuls write SM directly.

Per step: r matmuls -> sigmoid -> (DVE transpose, mul with hT) -> z
matmuls -> hc matmuls (stationary r*h in ST) -> tanh -> blend in SM bf16
-> one DVE transpose of h. The sigmoid/tanh/blend chains are split in 3
free-dim parts so downstream matmuls start as soon as their k-tiles are
ready. The x @ Wx precompute (bf16, N=512 matmuls) is interleaved into
the post-candidate bubble, 2 thunks/step, writing xpart chunks to DRAM
32 steps ahead of the scan; this both hides the precompute and keeps the
PE HAM clock-gate warm.
"""

import os
import sys
from contextlib import ExitStack

for _p in ("/opt/trn_rl_repo",):
    if os.path.isdir(_p) and _p not in sys.path:
        sys.path.insert(0, _p)

import numpy as np
import ml_dtypes

import concourse.bass as bass
import concourse.mybir as mybir
import concourse.tile as tile
from concourse import bacc
from concourse.bass import ds
from concourse.bass_utils import run_bass_kernel_spmd

B, S, I, H, O = 64, 512, 512, 1024, 2
NCORES = 8
BL = B // NCORES          # 8 batch rows per core
G3 = 3 * H                # 3072 gate features, gate order [r | z | hc]
KT = H // 128             # 8 k-tiles over hidden dim
KTI = I // 128            # 4 k-tiles over input dim
F32, BF16 = mybir.dt.float32, mybir.dt.bfloat16
AFT = mybir.ActivationFunctionType
PAD_CHUNKS = 2            # precompute runs 2 chunks (32 steps) ahead
PARTS = [(0, 64), (64, 128), (128, 256)]  # free-dim pipeline splits


def _pcol():
    """SM column permutation: position g*256+32m+i holds gate feat 128m+32g+i."""
    p = np.empty(H, np.int64)
    for g in range(4):
        for m in range(8):
            p[g * 256 + 32 * m + np.arange(32)] = 128 * m + 32 * g + np.arange(32)
    return p


def build(n_bodies=S // 16, num_devices=NCORES):
    """Build the Bass program. n_bodies 16-step bodies (32 for the real run)."""
    nsteps = 16 * n_bodies
    n_rows = BL * nsteps
    pad_rows = 128 * PAD_CHUNKS

    nc = bacc.Bacc("TRN2", target_bir_lowering=False, debug=False,
                   num_devices=num_devices)

    xt_d = nc.dram_tensor("xt", [I, n_rows + pad_rows], BF16, kind="ExternalInput")
    whh_d = nc.dram_tensor("whh", [128, KT * G3], BF16, kind="ExternalInput")
    wx_d = nc.dram_tensor("wx", [128, KTI * G3], BF16, kind="ExternalInput")
    bias_d = nc.dram_tensor("bias", [1, G3], BF16, kind="ExternalInput")
    h0sm_d = nc.dram_tensor("h0sm", [128, 256], BF16, kind="ExternalInput")
    h0st_d = nc.dram_tensor("h0st", [128, 256], BF16, kind="ExternalInput")
    id8_d = nc.dram_tensor("id8", [8, 32], BF16, kind="ExternalInput")
    ones1_d = nc.dram_tensor("ones1", [1, 128], BF16, kind="ExternalInput")
    wfc_d = nc.dram_tensor("wfc", [128, KT * O], BF16, kind="ExternalInput")
    bfc_d = nc.dram_tensor("bfc", [1, O], BF16, kind="ExternalInput")
    out_d = nc.dram_tensor("out", [BL, O], F32, kind="ExternalOutput")

    xpart_d = nc.dram_tensor("xpart", [n_rows + pad_rows, G3], BF16)

    with tile.TileContext(nc) as tc, ExitStack() as ctx:
        # ---------------- constants resident in SBUF ----------------
        pconst = ctx.enter_context(tc.tile_pool(name="pconst", bufs=1))
        whh = pconst.tile([128, KT * G3], BF16)
        for k in range(KT):
            nc.sync.dma_start(out=whh[:, G3 * k : G3 * (k + 1)],
                              in_=whh_d[:, G3 * k : G3 * (k + 1)])
        wx = pconst.tile([128, KTI * G3], BF16)
        for k in range(KTI):
            nc.sync.dma_start(out=wx[:, G3 * k : G3 * (k + 1)],
                              in_=wx_d[:, G3 * k : G3 * (k + 1)])
        bias_sb = pconst.tile([1, G3], BF16)
        nc.sync.dma_start(out=bias_sb, in_=bias_d[:, :])
        id8 = pconst.tile([8, 32], BF16)
        nc.sync.dma_start(out=id8, in_=id8_d[:, :])
        ones1 = pconst.tile([1, 128], BF16)
        nc.sync.dma_start(out=ones1, in_=ones1_d[:, :])
        wfc_sb = pconst.tile([128, KT * O], BF16)
        nc.sync.dma_start(out=wfc_sb, in_=wfc_d[:, :])
        bfc_sb = pconst.tile([1, O], BF16)
        nc.sync.dma_start(out=bfc_sb, in_=bfc_d[:, :])

        # persistent scan state
        hA = pconst.tile([128, 256], BF16)   # h in SM space (even steps in)
        nc.sync.dma_start(out=hA, in_=h0sm_d[:, :])
        hB = pconst.tile([128, 256], BF16)
        hT = pconst.tile([128, 256], BF16)   # h in ST space (matmul stationary)
        nc.sync.dma_start(out=hT, in_=h0st_d[:, :])

        # ---------------- pools ----------------
        pxp = ctx.enter_context(tc.tile_pool(name="pxp", bufs=3))
        pxt = ctx.enter_context(tc.tile_pool(name="pxt", bufs=2))
        pchunk = ctx.enter_context(tc.tile_pool(name="pchunk", bufs=2))
        ptmp = ctx.enter_context(tc.tile_pool(name="ptmp", bufs=1))
        pps = ctx.enter_context(tc.tile_pool(name="pps", bufs=1, space="PSUM"))
        ppps = ctx.enter_context(tc.tile_pool(name="ppps", bufs=2, space="PSUM"))

        r_ps = pps.tile([128, 512], F32, tag="r_ps")
        z_ps = pps.tile([128, 512], F32, tag="z_ps")
        hc_ps = pps.tile([128, 512], F32, tag="hc_ps")

        # ---------------- precompute chunk thunks ----------------
        def make_chunk_thunks(row_expr):
            """Emit thunks computing xpart rows [row_expr, row_expr+128).

            pe thunks: 1 dma + 12 matmul groups (drain ~2/step).
            act thunks: 6 psum->sbuf copies + 1 dma out (drain 1/step, u>=2).
            """
            st = {}

            def dma_xt():
                t = pxt.tile([128, KTI, 128], BF16, tag="xt")
                for k in range(KTI):
                    nc.sync.dma_start(
                        out=t[:, k, :],
                        in_=xt_d[128 * k : 128 * (k + 1), ds(row_expr, 128)])
                st["xt"] = t
                xpc = pchunk.tile([128, G3], BF16, tag="xpc")
                st["xpc"] = xpc
                st["pp"] = {}

            pe = [dma_xt]
            for n in range(6):
                def mm_a(n=n):
                    pp = ppps.tile([128, 512], F32, tag="pp")
                    st["pp"][n] = pp
                    cs = slice(512 * n, 512 * (n + 1))
                    nc.tensor.matmul(pp, ones1, bias_sb[:, cs],
                                     start=True, stop=False)
                    for k in range(2):
                        nc.tensor.matmul(
                            pp, st["xt"][:, k, :],
                            wx[:, G3 * k + 512 * n : G3 * k + 512 * (n + 1)],
                            start=False, stop=False)

                def mm_b(n=n):
                    pp = st["pp"][n]
                    for k in range(2, KTI):
                        nc.tensor.matmul(
                            pp, st["xt"][:, k, :],
                            wx[:, G3 * k + 512 * n : G3 * k + 512 * (n + 1)],
                            start=False, stop=(k == KTI - 1))

                pe += [mm_a, mm_b]

            act = []
            for n in range(6):
                def cp(n=n):
                    nc.scalar.copy(st["xpc"][:, 512 * n : 512 * (n + 1)],
                                   st["pp"][n])
                act.append(cp)

            def dma_out():
                nc.sync.dma_start(out=xpart_d[ds(row_expr, 128), :],
                                  in_=st["xpc"])
            act.append(dma_out)
            return pe, act

        # ---------------- one scan step ----------------
        def mm_init(gt, ps, xpf):
            for g in range(4):
                nc.tensor.matmul(
                    ps[32 * g : 32 * g + 32, :256], id8,
                    xpf[:, 1024 * gt + 256 * g : 1024 * gt + 256 * (g + 1)],
                    start=True, stop=False, tile_position=(0, 32 * g),
                    skip_group_check=True)

        def mm_gate(gt, ps, statT):
            for kc in range(KT):
                for g in range(4):
                    nc.tensor.matmul(
                        ps[32 * g : 32 * g + 32, :256],
                        statT[:, 32 * kc : 32 * kc + 32],
                        whh[:, G3 * kc + 1024 * gt + 256 * g :
                            G3 * kc + 1024 * gt + 256 * (g + 1)],
                        start=False, stop=(kc == KT - 1),
                        tile_position=(0, 32 * g), skip_group_check=True)

        def emit_step(u, row_expr, pe_fill, act_thunk):
            hprev, hnew = (hA, hB) if u % 2 == 0 else (hB, hA)

            xp = pxp.tile([8, G3], BF16, tag="xp")
            nc.sync.dma_start(out=xp, in_=xpart_d[ds(row_expr, 8), :])

            # bubble fill: r/z inits + precompute thunks
            mm_init(0, r_ps, xp)
            mm_init(1, z_ps, xp)
            for _ in range(2):
                if pe_fill:
                    pe_fill.pop(0)()

            mm_gate(0, r_ps, hT)

            sr = ptmp.tile([128, 256], BF16, tag="sr")
            rt = ptmp.tile([128, 256], BF16, tag="rt")
            rh = ptmp.tile([128, 256], BF16, tag="rh")
            for a, b in PARTS:
                nc.scalar.activation(sr[:, a:b], r_ps[:, a:b], AFT.Sigmoid)
            for a, b in PARTS:
                nc.vector.transpose(rt[:, a:b], sr[:, a:b])
                nc.vector.tensor_mul(rh[:, a:b], rt[:, a:b], hT[:, a:b])

            mm_init(2, hc_ps, xp)
            mm_gate(1, z_ps, hT)

            zsm = ptmp.tile([128, 256], BF16, tag="zsm")
            nc.scalar.activation(zsm, z_ps[:, :256], AFT.Sigmoid)

            mm_gate(2, hc_ps, rh)

            hcs = ptmp.tile([128, 256], BF16, tag="hcs")
            for a, b in PARTS:
                nc.scalar.activation(hcs[:, a:b], hc_ps[:, a:b], AFT.Tanh)
            if act_thunk is not None:
                act_thunk()

            dd = ptmp.tile([128, 256], BF16, tag="dd")
            uu = ptmp.tile([128, 256], BF16, tag="uu")
            for a, b in PARTS:
                nc.vector.tensor_sub(dd[:, a:b], hcs[:, a:b], hprev[:, a:b])
                nc.vector.tensor_mul(uu[:, a:b], zsm[:, a:b], dd[:, a:b])
                nc.vector.tensor_add(hnew[:, a:b], hprev[:, a:b], uu[:, a:b])
                nc.vector.transpose(hT[:, a:b], hnew[:, a:b])

        # ---------------- prefix: chunks 0, 1 ----------------
        for c in range(PAD_CHUNKS):
            pe, act = make_chunk_thunks(128 * c)
            pe[0]()
            for n in range(6):
                pe[1 + 2 * n]()
                pe[2 + 2 * n]()
                act[n]()
            act[6]()

        # ---------------- scan ----------------
        with tc.For_i(0, n_rows, 128) as iv:
            pe_fill, act_fill = make_chunk_thunks(iv + 128 * PAD_CHUNKS)
            # copy n lands at u=n+1: after chunk mm_b(n) (same step, earlier
            # position) and before chunk mm_a(n+2) (step n+2) so the psum
            # pool rotation sees all readers emitted.
            act_sched = {n + 1: act_fill[n] for n in range(len(act_fill))}
            for u in range(16):
                emit_step(u, iv + 8 * u, pe_fill, act_sched.get(u))
            assert not pe_fill

        # ---------------- FC head + log_softmax ----------------
        hrelu = ptmp.tile([128, 256], BF16, tag="hrelu")
        nc.scalar.activation(hrelu, hT, AFT.Relu)

        fc_ps = pps.tile([BL, O], F32, tag="fc")
        nc.tensor.matmul(fc_ps, ones1[:, :BL], bfc_sb, start=True, stop=False)
        for kc in range(KT):
            nc.tensor.matmul(fc_ps, hrelu[:, 32 * kc : 32 * kc + BL],
                             wfc_sb[:, O * kc : O * (kc + 1)],
                             start=False, stop=(kc == KT - 1))

        mx = ptmp.tile([BL, 1], F32, tag="mx")
        nc.vector.tensor_reduce(mx, fc_ps, mybir.AxisListType.X,
                                mybir.AluOpType.max)
        tt = ptmp.tile([BL, O], F32, tag="tt")
        nc.vector.tensor_scalar(tt, fc_ps, mx, None, mybir.AluOpType.subtract)
        ex = ptmp.tile([BL, O], F32, tag="ex")
        nc.scalar.activation(ex, tt, AFT.Exp)
        sm = ptmp.tile([BL, 1], F32, tag="sm")
        nc.vector.tensor_reduce(sm, ex, mybir.AxisListType.X,
                                mybir.AluOpType.add)
        lsm = ptmp.tile([BL, 1], F32, tag="lsm")
        nc.scalar.activation(lsm, sm, AFT.Ln)
        res = ptmp.tile([BL, O], F32, tag="res")
        nc.vector.tensor_scalar(res, tt, lsm, None, mybir.AluOpType.subtract)
        nc.sync.dma_start(out=out_d[:, :], in_=res)

    nc.compile()
    return nc


def prep_inputs(x, h, Wz, bz, Wr, br, Wh, bh, Wfc, bfc, nsteps=S):
    """Host-side prep: shard + relayout. Returns per-core input maps."""
    f32, bf16 = np.float32, ml_dtypes.bfloat16
    x = np.asarray(x, f32)[:, :nsteps, :]
    h0 = np.asarray(h, f32)[:, 0, :]
    pcol = _pcol()
    pad_rows = 128 * PAD_CHUNKS

    gates_h = [np.asarray(Wr, f32)[I:], np.asarray(Wz, f32)[I:],
               np.asarray(Wh, f32)[I:]]
    gates_x = [np.asarray(Wr, f32)[:I], np.asarray(Wz, f32)[:I],
               np.asarray(Wh, f32)[:I]]
    gates_b = [np.asarray(br, f32), np.asarray(bz, f32), np.asarray(bh, f32)]

    whh_img = np.zeros((128, KT * G3), bf16)
    for kc in range(KT):
        for gt in range(3):
            whh_img[:, G3 * kc + 1024 * gt : G3 * kc + 1024 * (gt + 1)] = \
                gates_h[gt][128 * kc : 128 * (kc + 1), pcol]
    wx_img = np.zeros((128, KTI * G3), bf16)
    for k in range(KTI):
        for gt in range(3):
            wx_img[:, G3 * k + 1024 * gt : G3 * k + 1024 * (gt + 1)] = \
                gates_x[gt][128 * k : 128 * (k + 1), pcol]
    bias_img = np.concatenate([g[pcol] for g in gates_b])[None, :].astype(bf16)

    id8 = np.zeros((8, 32), bf16)
    np.fill_diagonal(id8[:, :8], 1)
    ones1 = np.ones((1, 128), bf16)
    wfc_img = np.asarray(Wfc, f32).reshape(KT, 128, O).transpose(1, 0, 2) \
        .reshape(128, KT * O).astype(bf16)
    bfc_img = np.asarray(bfc, f32)[None, :].astype(bf16)

    in_maps = []
    for c in range(NCORES):
        xc = x[c * BL : (c + 1) * BL]                      # [8, S', I]
        xt = np.zeros((I, BL * nsteps + pad_rows), bf16)
        xt[:, : BL * nsteps] = xc.transpose(2, 1, 0).reshape(I, nsteps * BL)
        h0c = h0[c * BL : (c + 1) * BL]                    # [8, H]
        hv = h0c.reshape(BL, 8, 4, 32)                     # [b, m, g, i]
        h0sm = np.zeros((128, 256), bf16)
        h0st = np.zeros((128, 256), bf16)
        for g in range(4):
            h0sm[32 * g : 32 * g + BL, :] = hv[:, :, g, :].reshape(BL, 256)
            zt = np.zeros((32, 8, 32), f32)
            zt[:, :, :BL] = hv[:, :, g, :].transpose(2, 1, 0)
            h0st[32 * g : 32 * g + 32, :] = zt.reshape(32, 256)
        in_maps.append({
            "xt": xt, "h0sm": h0sm, "h0st": h0st,
            "whh": whh_img, "wx": wx_img, "bias": bias_img,
            "id8": id8, "ones1": ones1,
            "wfc": wfc_img, "bfc": bfc_img,
        })
    return in_maps


_BUILT = {}
_LAST_RESULTS = None


def kernel(**inputs):
    global _LAST_RESULTS
    key = "full"
    if key not in _BUILT:
        _BUILT[key] = build()
    nc = _BUILT[key]
    in_maps = prep_inputs(**inputs)
    trace = bool(int(os.environ.get("BASS_TRACE", "0") or "0"))
    res = run_bass_kernel_spmd(nc, in_maps, list(range(NCORES)), trace=trace)
    _LAST_RESULTS = res
    outs = [res.results[c]["out"] for c in range(NCORES)]
    return np.concatenate(outs, axis=0).astype(np.float32)


if __name__ == "__main__":
    np.random.seed(0)
    print("building...")
    nc = build(2, num_devices=1)
    print("build ok:", nc)


# revision 12
# speedup vs baseline: 1.7928x; 1.1532x over previous
"""GRU scan kernel for Trainium2, 8-core data-parallel.

Problem: B=64, S=512, I=512, H=1024, O=2 GRU + FC + log_softmax.

Strategy (v2): shard batch 8-way (8 rows/core). Per core, a 512-step scan
where each step streams Whh (bf16, [1024, 3072]) through the PE at 4-way
column-group concurrency (tile_position), with batch-8 stationaries.

Key layout: the "staircase" SM/ST pair, chosen so SM -> ST is exactly the
DVE's 32x32-block transpose (nc.vector.transpose):
  SM[32g+b, 32m+i] = v[b, 128m+32g+i]   (batch-major, for elementwise)
  ST[32g+i, 32m+b] = v[b, 128m+32g+i]   (feature-major; ST[:, 32k:32k+32]
                                          is the matmul stationary for
                                          contraction k-tile k)
Weights are column-permuted on the host so gate matmuls write SM directly.

Per step: r matmuls -> sigmoid -> (DVE transpose, mul with hT) -> z
matmuls -> hc matmuls (stationary r*h in ST) -> tanh -> blend in SM bf16
-> one DVE transpose of h. The sigmoid/tanh/blend chains are split in 3
free-dim parts so downstream matmuls start as soon as their k-tiles are
ready. The x @ Wx precompute (bf16, N=512 matmuls) is interleaved into
the post-candidate bubble, 2 thunks/step, writing xpart chunks to DRAM
32 steps ahead of the scan; this both hides the precompute and keeps the
PE HAM clock-gate warm.
"""

import os
import sys
from contextlib import ExitStack

for _p in ("/opt/trn_rl_repo",):
    if os.path.isdir(_p) and _p not in sys.path:
        sys.path.insert(0, _p)

import numpy as np
import ml_dtypes

import concourse.bass as bass
import concourse.mybir as mybir
import concourse.tile as tile
from concourse import bacc
from concourse.bass import ds
from concourse.bass_utils import run_bass_kernel_spmd

B, S, I, H, O = 64, 512, 512, 1024, 2
NCORES = 8
BL = B // NCORES          # 8 batch rows per core
G3 = 3 * H                # 3072 gate features, gate order [r | z | hc]
KT = H // 128             # 8 k-tiles over hidden dim
KTI = I // 128            # 4 k-tiles over input dim
F32, BF16 = mybir.dt.float32, mybir.dt.bfloat16
AFT = mybir.ActivationFunctionType
PAD_CHUNKS = 2            # precompute runs 2 chunks (32 steps) ahead
PARTS = [(0, 128), (128, 256)]  # free-dim pipeline splits


def _pcol():
    """SM column permutation: position g*256+32m+i holds gate feat 128m+32g+i."""
    p = np.empty(H, np.int64)
    for g in range(4):
        for m in range(8):
            p[g * 256 + 32 * m + np.arange(32)] = 128 * m + 32 * g + np.arange(32)
    return p


def build(n_bodies=S // 16, num_devices=NCORES):
    """Build the Bass program. n_bodies 16-step bodies (32 for the real run)."""
    nsteps = 16 * n_bodies
    n_rows = BL * nsteps
    pad_rows = 128 * PAD_CHUNKS

    nc = bacc.Bacc("TRN2", target_bir_lowering=False, debug=False,
                   num_devices=num_devices)

    xt_d = nc.dram_tensor("xt", [I, n_rows + pad_rows], BF16, kind="ExternalInput")
    whh_d = nc.dram_tensor("whh", [128, KT * G3], BF16, kind="ExternalInput")
    wx_d = nc.dram_tensor("wx", [128, KTI * G3], BF16, kind="ExternalInput")
    bias_d = nc.dram_tensor("bias", [1, G3], BF16, kind="ExternalInput")
    h0sm_d = nc.dram_tensor("h0sm", [128, 256], BF16, kind="ExternalInput")
    h0st_d = nc.dram_tensor("h0st", [128, 256], BF16, kind="ExternalInput")
    id8_d = nc.dram_tensor("id8", [8, 32], BF16, kind="ExternalInput")
    ones1_d = nc.dram_tensor("ones1", [1, 128], BF16, kind="ExternalInput")
    wfc_d = nc.dram_tensor("wfc", [128, KT * O], BF16, kind="ExternalInput")
    bfc_d = nc.dram_tensor("bfc", [1, O], BF16, kind="ExternalInput")
    out_d = nc.dram_tensor("out", [BL, O], F32, kind="ExternalOutput")

    xpart_d = nc.dram_tensor("xpart", [n_rows + pad_rows, G3], BF16)

    with tile.TileContext(nc) as tc, ExitStack() as ctx:
        # ---------------- constants resident in SBUF ----------------
        pconst = ctx.enter_context(tc.tile_pool(name="pconst", bufs=1))
        whh = pconst.tile([128, KT * G3], BF16)
        for k in range(KT):
            nc.sync.dma_start(out=whh[:, G3 * k : G3 * (k + 1)],
                              in_=whh_d[:, G3 * k : G3 * (k + 1)])
        wx = pconst.tile([128, KTI * G3], BF16)
        for k in range(KTI):
            nc.sync.dma_start(out=wx[:, G3 * k : G3 * (k + 1)],
                              in_=wx_d[:, G3 * k : G3 * (k + 1)])
        bias_sb = pconst.tile([1, G3], BF16)
        nc.sync.dma_start(out=bias_sb, in_=bias_d[:, :])
        id8 = pconst.tile([8, 32], BF16)
        nc.sync.dma_start(out=id8, in_=id8_d[:, :])
        ones1 = pconst.tile([1, 128], BF16)
        nc.sync.dma_start(out=ones1, in_=ones1_d[:, :])
        wfc_sb = pconst.tile([128, KT * O], BF16)
        nc.sync.dma_start(out=wfc_sb, in_=wfc_d[:, :])
        bfc_sb = pconst.tile([1, O], BF16)
        nc.sync.dma_start(out=bfc_sb, in_=bfc_d[:, :])

        # persistent scan state
        hA = pconst.tile([128, 256], BF16)   # h in SM space (even steps in)
        nc.sync.dma_start(out=hA, in_=h0sm_d[:, :])
        hB = pconst.tile([128, 256], BF16)
        hT = pconst.tile([128, 256], BF16)   # h in ST space (matmul stationary)
        nc.sync.dma_start(out=hT, in_=h0st_d[:, :])

        # ---------------- pools ----------------
        pxp = ctx.enter_context(tc.tile_pool(name="pxp", bufs=3))
        pxt = ctx.enter_context(tc.tile_pool(name="pxt", bufs=2))
        pchunk = ctx.enter_context(tc.tile_pool(name="pchunk", bufs=2))
        ptmp = ctx.enter_context(tc.tile_pool(name="ptmp", bufs=1))
        pps = ctx.enter_context(tc.tile_pool(name="pps", bufs=1, space="PSUM"))
        ppps = ctx.enter_context(tc.tile_pool(name="ppps", bufs=2, space="PSUM"))

        r_ps = pps.tile([128, 512], F32, tag="r_ps")
        z_ps = pps.tile([128, 512], F32, tag="z_ps")
        hc_ps = pps.tile([128, 512], F32, tag="hc_ps")

        # ---------------- precompute chunk thunks ----------------
        def make_chunk_thunks(row_expr):
            """Emit thunks computing xpart rows [row_expr, row_expr+128).

            pe thunks: 1 dma + 12 matmul groups (drain ~2/step).
            act thunks: 6 psum->sbuf copies + 1 dma out (drain 1/step, u>=2).
            """
            st = {}

            def dma_xt():
                t = pxt.tile([128, KTI, 128], BF16, tag="xt")
                for k in range(KTI):
                    nc.sync.dma_start(
                        out=t[:, k, :],
                        in_=xt_d[128 * k : 128 * (k + 1), ds(row_expr, 128)])
                st["xt"] = t
                xpc = pchunk.tile([128, G3], BF16, tag="xpc")
                st["xpc"] = xpc
                st["pp"] = {}

            pe = [dma_xt]
            for n in range(6):
                def mm_a(n=n):
                    pp = ppps.tile([128, 512], F32, tag="pp")
                    st["pp"][n] = pp
                    cs = slice(512 * n, 512 * (n + 1))
                    nc.tensor.matmul(pp, ones1, bias_sb[:, cs],
                                     start=True, stop=False)
                    nc.tensor.matmul(
                        pp, st["xt"][:, 0, :],
                        wx[:, 512 * n : 512 * (n + 1)],
                        start=False, stop=False)

                def mm_b(n=n):
                    pp = st["pp"][n]
                    for k in range(1, 3):
                        nc.tensor.matmul(
                            pp, st["xt"][:, k, :],
                            wx[:, G3 * k + 512 * n : G3 * k + 512 * (n + 1)],
                            start=False, stop=False)

                def mm_c(n=n):
                    pp = st["pp"][n]
                    k = KTI - 1
                    nc.tensor.matmul(
                        pp, st["xt"][:, k, :],
                        wx[:, G3 * k + 512 * n : G3 * k + 512 * (n + 1)],
                        start=False, stop=True)

                pe += [mm_a, mm_b, mm_c]

            # act schedule: copy n at u = 2 + ceil(3n/2); dma out at u=11
            act = {}
            for n in range(6):
                def cp(n=n):
                    nc.scalar.copy(st["xpc"][:, 512 * n : 512 * (n + 1)],
                                   st["pp"][n])
                act[2 + (3 * n + 1) // 2] = cp

            def dma_out():
                nc.sync.dma_start(out=xpart_d[ds(row_expr, 128), :],
                                  in_=st["xpc"])
            act[11] = dma_out
            return pe, act

        # ---------------- one scan step ----------------
        def mm_init(gt, ps, xpf):
            for g in range(4):
                nc.tensor.matmul(
                    ps[32 * g : 32 * g + 32, :256], id8,
                    xpf[:, 1024 * gt + 256 * g : 1024 * gt + 256 * (g + 1)],
                    start=True, stop=False, tile_position=(0, 32 * g),
                    skip_group_check=True)

        def mm_gate(gt, ps, statT):
            for kc in range(KT):
                for g in range(4):
                    nc.tensor.matmul(
                        ps[32 * g : 32 * g + 32, :256],
                        statT[:, 32 * kc : 32 * kc + 32],
                        whh[:, G3 * kc + 1024 * gt + 256 * g :
                            G3 * kc + 1024 * gt + 256 * (g + 1)],
                        start=False, stop=(kc == KT - 1),
                        tile_position=(0, 32 * g), skip_group_check=True)

        def emit_step(u, row_expr, pe_fill, act_thunk):
            hprev, hnew = (hA, hB) if u % 2 == 0 else (hB, hA)

            xp = pxp.tile([8, G3], BF16, tag="xp")
            nc.sync.dma_start(out=xp, in_=xpart_d[ds(row_expr, 8), :])

            # bubble fill: r/z inits + precompute thunks
            mm_init(0, r_ps, xp)
            mm_init(1, z_ps, xp)
            for _ in range(2):
                if pe_fill:
                    pe_fill.pop(0)()

            mm_gate(0, r_ps, hT)

            sr = ptmp.tile([128, 256], BF16, tag="sr")
            rt = ptmp.tile([128, 256], BF16, tag="rt")
            rh = ptmp.tile([128, 256], BF16, tag="rh")
            for a, b in PARTS:
                nc.scalar.activation(sr[:, a:b], r_ps[:, a:b], AFT.Sigmoid)
            for a, b in PARTS:
                nc.vector.transpose(rt[:, a:b], sr[:, a:b])
                nc.vector.tensor_mul(rh[:, a:b], rt[:, a:b], hT[:, a:b])

            mm_init(2, hc_ps, xp)
            mm_gate(1, z_ps, hT)

            zsm = ptmp.tile([128, 256], BF16, tag="zsm")
            nc.scalar.activation(zsm, z_ps[:, :256], AFT.Sigmoid)

            mm_gate(2, hc_ps, rh)

            hcs = ptmp.tile([128, 256], BF16, tag="hcs")
            for a, b in PARTS:
                nc.scalar.activation(hcs[:, a:b], hc_ps[:, a:b], AFT.Tanh)
            if act_thunk is not None:
                act_thunk()

            dd = ptmp.tile([128, 256], BF16, tag="dd")
            uu = ptmp.tile([128, 256], BF16, tag="uu")
            for a, b in PARTS:
                nc.vector.tensor_sub(dd[:, a:b], hcs[:, a:b], hprev[:, a:b])
                nc.vector.tensor_mul(uu[:, a:b], zsm[:, a:b], dd[:, a:b])
                nc.vector.tensor_add(hnew[:, a:b], hprev[:, a:b], uu[:, a:b])
                nc.vector.transpose(hT[:, a:b], hnew[:, a:b])

        # ---------------- prefix: chunks 0, 1 ----------------
        for c in range(PAD_CHUNKS):
            pe, act = make_chunk_thunks(128 * c)
            pe[0]()
            acts = [act[k] for k in sorted(act)]
            for n in range(6):
                pe[1 + 3 * n]()
                pe[2 + 3 * n]()
                pe[3 + 3 * n]()
                acts[n]()
            acts[6]()

        # ---------------- scan ----------------
        with tc.For_i(0, n_rows, 128) as iv:
            pe_fill, act_sched = make_chunk_thunks(iv + 128 * PAD_CHUNKS)
            for u in range(16):
                emit_step(u, iv + 8 * u, pe_fill, act_sched.get(u))
            assert not pe_fill

        # ---------------- FC head + log_softmax ----------------
        hrelu = ptmp.tile([128, 256], BF16, tag="hrelu")
        nc.scalar.activation(hrelu, hT, AFT.Relu)

        fc_ps = pps.tile([BL, O], F32, tag="fc")
        nc.tensor.matmul(fc_ps, ones1[:, :BL], bfc_sb, start=True, stop=False)
        for kc in range(KT):
            nc.tensor.matmul(fc_ps, hrelu[:, 32 * kc : 32 * kc + BL],
                             wfc_sb[:, O * kc : O * (kc + 1)],
                             start=False, stop=(kc == KT - 1))

        mx = ptmp.tile([BL, 1], F32, tag="mx")
        nc.vector.tensor_reduce(mx, fc_ps, mybir.AxisListType.X,
                                mybir.AluOpType.max)
        tt = ptmp.tile([BL, O], F32, tag="tt")
        nc.vector.tensor_scalar(tt, fc_ps, mx, None, mybir.AluOpType.subtract)
        ex = ptmp.tile([BL, O], F32, tag="ex")
        nc.scalar.activation(ex, tt, AFT.Exp)
        sm = ptmp.tile([BL, 1], F32, tag="sm")
        nc.vector.tensor_reduce(sm, ex, mybir.AxisListType.X,
                                mybir.AluOpType.add)
        lsm = ptmp.tile([BL, 1], F32, tag="lsm")
        nc.scalar.activation(lsm, sm, AFT.Ln)
        res = ptmp.tile([BL, O], F32, tag="res")
        nc.vector.tensor_scalar(res, tt, lsm, None, mybir.AluOpType.subtract)
        nc.sync.dma_start(out=out_d[:, :], in_=res)

    nc.compile()
    return nc


def prep_inputs(x, h, Wz, bz, Wr, br, Wh, bh, Wfc, bfc, nsteps=S):
    """Host-side prep: shard + relayout. Returns per-core input maps."""
    f32, bf16 = np.float32, ml_dtypes.bfloat16
    x = np.asarray(x, f32)[:, :nsteps, :]
    h0 = np.asarray(h, f32)[:, 0, :]
    pcol = _pcol()
    pad_rows = 128 * PAD_CHUNKS

    gates_h = [np.asarray(Wr, f32)[I:], np.asarray(Wz, f32)[I:],
               np.asarray(Wh, f32)[I:]]
    gates_x = [np.asarray(Wr, f32)[:I], np.asarray(Wz, f32)[:I],
               np.asarray(Wh, f32)[:I]]
    gates_b = [np.asarray(br, f32), np.asarray(bz, f32), np.asarray(bh, f32)]

    whh_img = np.zeros((128, KT * G3), bf16)
    for kc in range(KT):
        for gt in range(3):
            whh_img[:, G3 * kc + 1024 * gt : G3 * kc + 1024 * (gt + 1)] = \
                gates_h[gt][128 * kc : 128 * (kc + 1), pcol]
    wx_img = np.zeros((128, KTI * G3), bf16)
    for k in range(KTI):
        for gt in range(3):
            wx_img[:, G3 * k + 1024 * gt : G3 * k + 1024 * (gt + 1)] = \
                gates_x[gt][128 * k : 128 * (k + 1), pcol]
    bias_img = np.concatenate([g[pcol] for g in gates_b])[None, :].astype(bf16)

    id8 = np.zeros((8, 32), bf16)
    np.fill_diagonal(id8[:, :8], 1)
    ones1 = np.ones((1, 128), bf16)
    wfc_img = np.asarray(Wfc, f32).reshape(KT, 128, O).transpose(1, 0, 2) \
        .reshape(128, KT * O).astype(bf16)
    bfc_img = np.asarray(bfc, f32)[None, :].astype(bf16)

    in_maps = []
    for c in range(NCORES):
        xc = x[c * BL : (c + 1) * BL]                      # [8, S', I]
        xt = np.zeros((I, BL * nsteps + pad_rows), bf16)
        xt[:, : BL * nsteps] = xc.transpose(2, 1, 0).reshape(I, nsteps * BL)
        h0c = h0[c * BL : (c + 1) * BL]                    # [8, H]
        hv = h0c.reshape(BL, 8, 4, 32)                     # [b, m, g, i]
        h0sm = np.zeros((128, 256), bf16)
        h0st = np.zeros((128, 256), bf16)
        for g in range(4):
            h0sm[32 * g : 32 * g + BL, :] = hv[:, :, g, :].reshape(BL, 256)
            zt = np.zeros((32, 8, 32), f32)
            zt[:, :, :BL] = hv[:, :, g, :].transpose(2, 1, 0)
            h0st[32 * g : 32 * g + 32, :] = zt.reshape(32, 256)
        in_maps.append({
            "xt": xt, "h0sm": h0sm, "h0st": h0st,
            "whh": whh_img, "wx": wx_img, "bias": bias_img,
            "id8": id8, "ones1": ones1,
            "wfc": wfc_img, "bfc": bfc_img,
        })
    return in_maps


_BUILT = {}
_LAST_RESULTS = None


def kernel(**inputs):
    global _LAST_RESULTS
    key = "full"
    if key not in _BUILT:
        _BUILT[key] = build()
    nc = _BUILT[key]
    in_maps = prep_inputs(**inputs)
    trace = bool(int(os.environ.get("BASS_TRACE", "0") or "0"))
    res = run_bass_kernel_spmd(nc, in_maps, list(range(NCORES)), trace=trace)
    _LAST_RESULTS = res
    outs = [res.results[c]["out"] for c in range(NCORES)]
    return np.concatenate(outs, axis=0).astype(np.float32)


if __name__ == "__main__":
    np.random.seed(0)
    print("building...")
    nc = build(2, num_devices=1)
    print("build ok:", nc)
